# revision 21
# baseline (speedup 1.0000x reference)
"""Trainium2 Bass kernel for causal self-attention with doubled rotary.

Full-input contract: kernel(**inputs) takes the complete tensors
(x [4,2048,2048], wq/wk/wv/wo [2048,2048]) and returns [4,2048,2048] fp32.

Sharding: 8 cores = 4 batch elements x 2 head-halves (8 heads each).
Each core computes a partial output projection (its heads' columns of wo);
the host sums the two partials per batch element.

All matmul operands are bf16 (fp8 exceeds the error budget on every path —
measured 2.8-4.7e-2 vs the 2e-2 gate; all-bf16 lands at ~3.6e-3). bf16
halves DMA bytes vs fp32r and enables FWL weight loads.

Per-core structure (engine streams execute in emission order; independent
work is interleaved at emission time to keep the PE dense):
  - phase 0: two sweeps over x panels (512-wide). Sweep A: q/k projections
    of group 0 + V projection (all 8 heads, low column half). Sweep B:
    V high half + doubled-angle rotary (R(t)^2 == R(2t)) for group 0.
  - phases 1..3: q/k projections + rotary of group g interleaved with
    attention of head pair g-1. Attention is computed transposed (ST[s,t])
    so exp(ST) feeds the PV matmul directly with v stationary.
  - softmax denominator: DVE accumulates the bf16 exp chunks into an f32r
    panel accumulator; ONE all-ones matmul per (head, panel) does the
    partition reduce + broadcast (vs one matmul per chunk).
  - y stays resident in SBUF (no DRAM spill); the output projection reads
    it directly, interleaved with the last attention pair.
"""

import os
import sys

for _p in ("/opt/trn_rl_repo", "/root/.axon_site/_ro/trn_rl_repo"):
    if os.path.isdir(_p) and _p not in sys.path:
        sys.path.insert(0, _p)

import numpy as np

import concourse.bass as bass
import concourse.mybir as mybir
from concourse import bacc
from concourse.bass import ds
from concourse.tile import TileContext
from concourse.bass_utils import run_bass_kernel_spmd

F32 = mybir.dt.float32
F32R = mybir.dt.float32r
BF16 = mybir.dt.bfloat16
FP16 = mybir.dt.float16

P = 128          # partitions / head dim
T = 2048         # sequence length
E = 2048         # embedding dim
B = 4
HPC = 8          # heads per core
D = 128          # head dim
PAN = 512        # panel width (PSUM bank limit for fp32)
NPAN = T // PAN  # 4
EO = E // P      # 16 contraction chunks for projections
EQ = 4           # eo chunks per input-DMA quarter
NGRP = 4         # head pairs per core
NCH = T // P     # 16 s-chunks (also v t-tiles)
SCALE = 1.0 / float(np.sqrt(D))
NEG = -1.0e9

ADD = mybir.AluOpType.add
MULT = mybir.AluOpType.mult
EXP = mybir.ActivationFunctionType.Exp


def _zip_emit(*lists):
    """Emit thunks from several lists round-robin, proportionally."""
    lists = [list(l) for l in lists if l]
    if not lists:
        return
    total = max(len(l) for l in lists)
    idx = [0.0] * len(lists)
    step = [len(l) / total for l in lists]
    for _ in range(total):
        for li, l in enumerate(lists):
            idx[li] += step[li]
            while idx[li] >= 1.0 and l:
                l.pop(0)()
                idx[li] -= 1.0
    for l in lists:
        for f in l:
            f()


class Ctx:
    pass


def _dma_quarters(nc, dst, src_re, eng=None):
    """Split a [P, EO, W] load into EO/EQ quarter DMAs for early starts."""
    eng = eng if eng is not None else nc.sync
    for qq in range(EO // EQ):
        eng.dma_start(
            dst[:, ds(qq * EQ, EQ), :], src_re[:, ds(qq * EQ, EQ), :]
        )


def build_program():
    nc = bacc.Bacc()
    cx = Ctx()
    cx.nc = nc

    cx.xT = nc.declare_dram_parameter("xT", [E, T], BF16, isOutput=False)
    cx.wqT = nc.declare_dram_parameter("wqT", [E, HPC * D], BF16, isOutput=False)
    cx.wkT = nc.declare_dram_parameter("wkT", [E, HPC * D], BF16, isOutput=False)
    cx.wvT = nc.declare_dram_parameter("wvT", [E, HPC * D], BF16, isOutput=False)
    cx.woT = nc.declare_dram_parameter("woT", [HPC * D, E], BF16, isOutput=False)
    cx.cos2 = nc.declare_dram_parameter("cos2", [P, T], FP16, isOutput=False)
    cx.sin2 = nc.declare_dram_parameter("sin2", [P, T], FP16, isOutput=False)
    cx.mask = nc.declare_dram_parameter("mask", [P, P], BF16, isOutput=False)
    cx.out = nc.declare_dram_parameter("out", [E, T], F32, isOutput=True)

    with TileContext(nc) as tc:
        cx.tc = tc
        with tc.tile_pool(name="const", bufs=1) as cpool:
            om_f = cpool.tile([P, P], F32, tag="om_f")
            nc.vector.memset(om_f, 1.0)
            cx.onesmat = cpool.tile([P, P], FP16, tag="onesmat")
            nc.scalar.copy(cx.onesmat, om_f)
            cx.mk = cpool.tile([P, P], BF16, tag="mk")

            with (
                tc.tile_pool(name="ex", bufs=6) as expool,
                tc.tile_pool(name="acc", bufs=3) as accpool,
                tc.tile_pool(name="dn", bufs=2) as dnpool,
                tc.tile_pool(name="qk", bufs=2) as qkpool,
                tc.tile_pool(name="vp", bufs=1) as vpool,
                tc.tile_pool(name="yp", bufs=1) as ypool,
                tc.tile_pool(name="psS", bufs=3, space="PSUM") as psS,
                tc.tile_pool(name="psY", bufs=2, space="PSUM") as psY,
            ):
                cx.expool, cx.accpool, cx.dnpool = expool, accpool, dnpool
                cx.qkpool = qkpool
                cx.v_sb = vpool.tile([P, NCH, HPC * D], BF16, tag="v")
                cx.y_sb = ypool.tile([P, HPC, T], BF16, tag="y")
                cx.psS, cx.psY = psS, psY
                cx.qkv = {}      # g -> (qT, kT)
                cx._w = {}       # g -> (wq_sb, wk_sb)
                cx._pstate = {}  # g -> {xj: xp tile}

                with (
                    tc.tile_pool(name="tab", bufs=1) as tabpool,
                    tc.tile_pool(name="xp", bufs=2) as xpool,
                    tc.tile_pool(name="wqk", bufs=2) as wqkpool,
                    tc.tile_pool(name="wv", bufs=1) as wvpool,
                    tc.tile_pool(name="rot", bufs=1) as rotpool,
                    tc.tile_pool(name="sw", bufs=2) as swpool,
                    tc.tile_pool(name="psP", bufs=3, space="PSUM") as psP,
                ):
                    cx.xpool, cx.wqkpool, cx.wvpool = xpool, wqkpool, wvpool
                    cx.rotpool, cx.swpool, cx.psP = rotpool, swpool, psP

                    # phase 0 sweep A: q/k of group 0 + v low half
                    for f in _proj_thunks(cx, 0, v_half=0):
                        f()

                    def load_tables():
                        cx.c2 = tabpool.tile([P, T], FP16, tag="c2")
                        nc.gpsimd.dma_start(cx.c2, cx.cos2[:, :])
                        cx.s2 = tabpool.tile([P, T], FP16, tag="s2")
                        nc.gpsimd.dma_start(cx.s2, cx.sin2[:, :])
                        nc.gpsimd.dma_start(cx.mk, cx.mask[:, :])

                    # phase 0 sweep B: v high half + rotary of group 0,
                    # with group 1's weights/panel prefetched behind it.
                    # Tables are emitted after sweep B's wv/x loads so the
                    # gpsimd DMA queue serves the v matmuls first.
                    vs = _vsweep_thunks(cx, v_half=1)
                    vs[0]()
                    vs[1]()
                    load_tables()
                    _zip_emit(
                        vs[2:] + _proj_prefetch(cx, 1),
                        _rot_thunks(cx, 0),
                    )

                    # phases 1..3 merged into ONE proportional zip so the
                    # scheduler always has projection matmuls available to
                    # hide the exp (ACT) latency of attention chunks — the
                    # per-phase version starved at every phase tail.
                    projall = []
                    attnall = []
                    for g in range(1, NGRP):
                        projall += _proj_thunks(cx, g) + _rot_thunks(cx, g)
                        if g + 1 < NGRP:
                            projall += _proj_prefetch(cx, g + 1)
                        attnall += _attn_thunks(cx, g - 1)
                    _zip_emit(projall, attnall)

                with (
                    tc.tile_pool(name="wo", bufs=1) as wopool,
                    tc.tile_pool(name="ob", bufs=3) as opool,
                    tc.tile_pool(name="psO", bufs=3, space="PSUM") as psO,
                ):
                    cx.opool, cx.psO = opool, psO
                    cx.wo_sb = wopool.tile([P, HPC, E], BF16, tag="wo")
                    # quartered, low e-columns first, so the first outproj
                    # e-tiles only wait on the first 1MB
                    for qq in range(4):
                        nc.gpsimd.dma_start(
                            cx.wo_sb[:, :, ds(qq * (E // 4), E // 4)],
                            cx.woT.rearrange("(c p) e -> p c e", p=P)[
                                :, :, ds(qq * (E // 4), E // 4)
                            ],
                        )
                    panels = [_attn_thunks(cx, NGRP - 1, only_jp=jp)
                              for jp in range(NPAN)]
                    oproj = [_outproj_thunks(cx, jp) for jp in range(NPAN)]
                    for f in panels[0]:
                        f()
                    for jp in range(1, NPAN):
                        _zip_emit(panels[jp], oproj[jp - 1])
                    for f in oproj[NPAN - 1]:
                        f()

    nc.finalize()
    return nc


def _load_panel(cx, xj, state):
    def f():
        xp = cx.xpool.tile([P, EO, PAN], BF16, tag="xp")
        _dma_quarters(
            cx.nc, xp,
            cx.xT.rearrange("(eo p) t -> p eo t", p=P)[:, :, ds(xj * PAN, PAN)],
        )
        state[xj] = xp
    return f


def _first_panel_interleaved(cx, g, state):
    """Phase-0 preamble: per-eo DMAs of x panel 0 interleaved with the
    q/k weight chunks so the first matmul chain starts within a few us."""
    nc = cx.nc
    xp = cx.xpool.tile([P, EO, PAN], BF16, tag="xp")
    state[0] = xp
    xsrc = cx.xT.rearrange("(eo p) t -> p eo t", p=P)[:, :, ds(0, PAN)]
    wq_sb = cx.wqkpool.tile([P, EO, 2 * D], BF16, tag="wq")
    wk_sb = cx.wqkpool.tile([P, EO, 2 * D], BF16, tag="wk")
    qsrc = cx.wqT.rearrange("(eo p) d -> p eo d", p=P)
    ksrc = cx.wkT.rearrange("(eo p) d -> p eo d", p=P)
    for eo in range(EO):
        nc.sync.dma_start(xp[:, ds(eo, 1), :], xsrc[:, ds(eo, 1), :])
        nc.gpsimd.dma_start(
            wq_sb[:, ds(eo, 1), :], qsrc[:, ds(eo, 1), ds(g * 2 * D, 2 * D)]
        )
        nc.gpsimd.dma_start(
            wk_sb[:, ds(eo, 1), :], ksrc[:, ds(eo, 1), ds(g * 2 * D, 2 * D)]
        )
    qT = cx.qkpool.tile([P, 2, T], BF16, tag="qT")
    kT = cx.qkpool.tile([P, 2, T], BF16, tag="kT")
    cx.qkv[g] = (qT, kT)
    cx._w[g] = (wq_sb, wk_sb)


def _load_wv_half(cx, half):
    def f():
        wv_sb = cx.wvpool.tile([P, EO, HPC * D // 2], BF16, tag="wv")
        _dma_quarters(
            cx.nc, wv_sb,
            cx.wvT.rearrange("(eo p) d -> p eo d", p=P)[
                :, :, ds(half * HPC * D // 2, HPC * D // 2)
            ],
            eng=cx.nc.gpsimd,
        )
        cx._wv = wv_sb
    return f


def _v_group(cx, state, xj, tt, half):
    """v for all 8 heads, one s-chunk, one 512-column half."""
    def f():
        nc = cx.nc
        xp = state[xj]
        ps = cx.psP.tile([P, PAN], F32, tag="psP")
        for eo in range(EO):
            nc.tensor.matmul(
                ps,
                lhsT=xp[:, eo, ds(tt * P, P)],
                rhs=cx._wv[:, eo, :],
                start=(eo == 0),
                stop=(eo == EO - 1),
            )
        nc.scalar.copy(
            cx.v_sb[:, xj * (PAN // P) + tt, ds(half * PAN, PAN)], ps
        )
    return f


def _proj_prefetch(cx, g):
    """Prefetch thunk for group g's weights + first x panel; emitted during
    the PREVIOUS phase so phase g starts with data in flight (the DMAs wait
    on buffer-slot semaphores, so early emission is always safe)."""
    nc = cx.nc
    state = cx._pstate.setdefault(g, {})

    def f():
        wq_sb = cx.wqkpool.tile([P, EO, 2 * D], BF16, tag="wq")
        _dma_quarters(
            nc, wq_sb,
            cx.wqT.rearrange("(eo p) d -> p eo d", p=P)[:, :, ds(g * 2 * D, 2 * D)],
            eng=nc.gpsimd,
        )
        wk_sb = cx.wqkpool.tile([P, EO, 2 * D], BF16, tag="wk")
        _dma_quarters(
            nc, wk_sb,
            cx.wkT.rearrange("(eo p) d -> p eo d", p=P)[:, :, ds(g * 2 * D, 2 * D)],
            eng=nc.gpsimd,
        )
        qT = cx.qkpool.tile([P, 2, T], BF16, tag="qT")
        kT = cx.qkpool.tile([P, 2, T], BF16, tag="kT")
        cx.qkv[g] = (qT, kT)
        cx._w[g] = (wq_sb, wk_sb)
        _load_panel(cx, 0, state)()

    return [f]


def _proj_thunks(cx, g, v_half=None):
    """Thunks for group g's q/k projections (+ v half during phase 0).

    For g == 0 the weights/panel-0 setup is emitted inline (interleaved
    per-eo DMAs); for g > 0 it happened in _proj_prefetch during the
    previous phase."""
    nc = cx.nc
    thunks = []
    state = cx._pstate.setdefault(g, {})

    if g == 0:
        thunks.append(lambda: _first_panel_interleaved(cx, g, state))
        if v_half is not None:
            thunks.append(_load_wv_half(cx, v_half))

    def qk_group(xj, wi, hl):
        def f():
            xp = state[xj]
            w_sb = cx._w[g][wi]
            dst = cx.qkv[g][wi]
            ps = cx.psP.tile([P, PAN], F32, tag="psP")
            for eo in range(EO):
                nc.tensor.matmul(
                    ps,
                    lhsT=w_sb[:, eo, ds(hl * D, D)],
                    rhs=xp[:, eo, :],
                    start=(eo == 0),
                    stop=(eo == EO - 1),
                )
            nc.vector.tensor_copy(dst[:, hl, ds(xj * PAN, PAN)], ps)
        return f

    for xj in range(NPAN):
        if xj + 1 < NPAN:
            thunks.append(_load_panel(cx, xj + 1, state))
        for wi in range(2):
            for hl in range(2):
                thunks.append(qk_group(xj, wi, hl))
        if v_half is not None:
            for tt in range(PAN // P):
                thunks.append(_v_group(cx, state, xj, tt, v_half))
    return thunks


def _vsweep_thunks(cx, v_half):
    """Second phase-0 sweep: reload x panels, compute the other v half."""
    thunks = []
    state = {}
    thunks.append(_load_wv_half(cx, v_half))
    thunks.append(_load_panel(cx, 0, state))
    for xj in range(NPAN):
        if xj + 1 < NPAN:
            thunks.append(_load_panel(cx, xj + 1, state))
        for tt in range(PAN // P):
            thunks.append(_v_group(cx, state, xj, tt, v_half))
    return thunks


def _rot_thunks(cx, g):
    """Doubled-angle rotary on group g's qT/kT, one 512-panel at a time."""
    nc = cx.nc
    thunks = []

    def rot_panel(src_i, hl, jp):
        def f():
            src = cx.qkv[g][src_i]
            sl = ds(jp * PAN, PAN)
            qsw = cx.swpool.tile([P, PAN], BF16, tag="qsw")
            nc.sync.dma_start(qsw[0:64, :], src[64:128, hl, sl])
            nc.sync.dma_start(qsw[64:128, :], src[0:64, hl, sl])
            tmp = cx.rotpool.tile([P, PAN], FP16, tag="rtmp")
            nc.vector.tensor_tensor(tmp, qsw[:, :], cx.s2[:, sl], op=MULT)
            nc.vector.tensor_tensor(
                src[:, hl, sl], src[:, hl, sl], cx.c2[:, sl], op=MULT
            )
            nc.vector.tensor_tensor(src[:, hl, sl], src[:, hl, sl], tmp, op=ADD)
        return f

    for jp in range(NPAN):
        for src_i in range(2):
            for hl in range(2):
                thunks.append(rot_panel(src_i, hl, jp))
    return thunks


def _attn_thunks(cx, g, only_jp=None):
    """Thunk list for the attention of head pair g (heads 2g, 2g+1)."""
    nc = cx.nc
    thunks = []
    st8 = cx.__dict__.setdefault(f"_attn_state_{g}", {})

    exst = {}

    def qk_part(hl, jp, i):
        def f():
            qT, kT = cx.qkv[g]
            if i == 0:
                ytp = cx.psY.tile([P, PAN], F32, tag="psY")
                acc = cx.accpool.tile([P, PAN], FP16, tag="acc")
                st8[(hl, jp)] = (ytp, acc)
            di = i - 4 * jp
            off = P * di if di > 0 else 0
            w = PAN - off
            st = cx.psS.tile([P, PAN], F32, tag="psS")
            stw = st[:, off:PAN]
            nc.tensor.matmul(
                stw,
                lhsT=kT[:, hl, ds(i * P, P)],
                rhs=qT[:, hl, ds(jp * PAN + off, w)],
                start=True,
                stop=True,
            )
            if di >= 0:
                nc.vector.tensor_tensor(
                    st[:, off:off + P], st[:, off:off + P], cx.mk, op=ADD
                )
            ex = cx.expool.tile([P, PAN], BF16, tag="ex")
            exw = ex[:, off:PAN]
            nc.scalar.activation(exw, stw, EXP, scale=SCALE)
            exst[(hl, i)] = (ex, exw, off)
        return f

    def pv_part(hl, jp, i):
        def f():
            nch = 4 * jp + 4
            ytp, acc = st8[(hl, jp)]
            ex, exw, off = exst.pop((hl, i))
            nc.tensor.matmul(
                ytp[:, off:PAN],
                lhsT=cx.v_sb[:, i, ds((2 * g + hl) * D, D)],
                rhs=exw,
                start=(i == 0),
                stop=(i == nch - 1),
            )
            if i == 0:
                nc.vector.tensor_copy(acc, ex)
            else:
                nc.vector.tensor_tensor(
                    acc[:, off:PAN], acc[:, off:PAN], exw, op=ADD
                )
        return f

    def finalize(hl, jp):
        def f():
            h = 2 * g + hl
            ytp, acc = st8.pop((hl, jp))
            dps = cx.psS.tile([P, PAN], F32, tag="psS")
            nc.tensor.matmul(
                dps, lhsT=cx.onesmat, rhs=acc, start=True, stop=True
            )
            rdb = cx.dnpool.tile([P, PAN], F32, tag="rdb")
            nc.vector.reciprocal_approx_fast(out=rdb, in_=dps)
            nc.vector.tensor_tensor(
                cx.y_sb[:, h, ds(jp * PAN, PAN)], ytp, rdb, op=MULT
            )
        return f

    jps = range(NPAN) if only_jp is None else [only_jp]
    for jp in jps:
        nch = 4 * jp + 4
        # one-chunk software pipeline: the QK+exp of chunk i+1 is emitted
        # before the PV of chunk i, so the PE always has a QK to run while
        # ACT computes the exp the next PV needs.
        for hl in range(2):
            thunks.append(qk_part(hl, jp, 0))
        for i in range(1, nch):
            for hl in range(2):
                thunks.append(qk_part(hl, jp, i))
                thunks.append(pv_part(hl, jp, i - 1))
        for hl in range(2):
            thunks.append(pv_part(hl, jp, nch - 1))
        for hl in range(2):
            thunks.append(finalize(hl, jp))
    return thunks


def _outproj_thunks(cx, jp):
    """Output projection for t-panel jp over all 16 e-tiles."""
    nc = cx.nc
    thunks = []

    def etile(et):
        def f():
            ps = cx.psO.tile([P, PAN], F32, tag="psO")
            for dc in range(HPC):
                nc.tensor.matmul(
                    ps,
                    lhsT=cx.wo_sb[:, dc, ds(et * P, P)],
                    rhs=cx.y_sb[:, dc, ds(jp * PAN, PAN)],
                    start=(dc == 0),
                    stop=(dc == HPC - 1),
                )
            ob = cx.opool.tile([P, PAN], F32, tag="ob")
            nc.scalar.copy(ob, ps)
            eng = cx.nc.gpsimd if et % 2 == 0 else cx.nc.scalar
            eng.dma_start(
                cx.out[ds(et * P, P), ds(jp * PAN, PAN)], ob
            )
        return f

    for et in range(2 * HPC):
        thunks.append(etile(et))
    return thunks


def make_tables():
    j = np.arange(0, D, 2, dtype=np.float64) / D
    inv_freq = 1.0 / (10000.0 ** j)
    t = np.arange(T, dtype=np.float64)
    fr = np.outer(t, inv_freq)                            # [T, 64]
    c2 = np.cos(2.0 * fr).T                               # [64, T]
    s2 = np.sin(2.0 * fr).T
    cos2 = np.concatenate([c2, c2], axis=0).astype(np.float16)
    sin2 = np.concatenate([s2, -s2], axis=0).astype(np.float16)
    return cos2, sin2


def make_mask():
    import ml_dtypes
    s = np.arange(P)[:, None]
    c = np.arange(P)[None, :]
    return np.where(s <= c, 0.0, NEG).astype(ml_dtypes.bfloat16)


def make_in_maps(x, wq, wk, wv, wo):
    import ml_dtypes
    bf = ml_dtypes.bfloat16
    cos2, sin2 = make_tables()
    mask = make_mask()
    in_maps = []
    for c in range(8):
        b, hh = c // 2, c % 2
        rows = slice(hh * HPC * D, (hh + 1) * HPC * D)
        in_maps.append({
            "xT": np.ascontiguousarray(x[b].T).astype(bf),
            "wqT": np.ascontiguousarray(wq[rows].T).astype(bf),
            "wkT": np.ascontiguousarray(wk[rows].T).astype(bf),
            "wvT": np.ascontiguousarray(wv[rows].T).astype(bf),
            "woT": np.ascontiguousarray(wo[:, rows].T).astype(bf),
            "cos2": cos2,
            "sin2": sin2,
            "mask": mask,
        })
    return in_maps


_PROGRAM_CACHE = {}


def get_program():
    if "nc" not in _PROGRAM_CACHE:
        _PROGRAM_CACHE["nc"] = build_program()
    return _PROGRAM_CACHE["nc"]


def kernel(x, wq, wk, wv, wo, _results_hook=None):
    x = np.asarray(x, dtype=np.float32)
    wq = np.asarray(wq, dtype=np.float32)
    wk = np.asarray(wk, dtype=np.float32)
    wv = np.asarray(wv, dtype=np.float32)
    wo = np.asarray(wo, dtype=np.float32)

    nc = get_program()
    in_maps = make_in_maps(x, wq, wk, wv, wo)
    res = run_bass_kernel_spmd(nc, in_maps, list(range(8)))
    if _results_hook is not None:
        _results_hook(res)
    outs = [r["out"] for r in res.results]
    full = np.empty((B, T, E), dtype=np.float32)
    for b in range(B):
        full[b] = (outs[2 * b] + outs[2 * b + 1]).T
    return full


# revision 25
# speedup vs baseline: 1.1748x; 1.1748x over previous
"""Trainium2 Bass kernel for causal self-attention with doubled rotary.

Full-input contract: kernel(**inputs) takes the complete tensors
(x [4,2048,2048], wq/wk/wv/wo [2048,2048]) and returns [4,2048,2048] fp32.

Sharding: 8 cores = 4 batch elements x 2 head-halves (8 heads each).
Each core computes a partial output projection (its heads' columns of wo);
the host sums the two partials per batch element.

All matmul operands are bf16 (fp8 exceeds the error budget on every path —
measured 2.8-4.7e-2 vs the 2e-2 gate; all-bf16 lands at ~3.6e-3). bf16
halves DMA bytes vs fp32r and enables FWL weight loads.

Per-core structure (engine streams execute in emission order; independent
work is interleaved at emission time to keep the PE dense):
  - phase 0: two sweeps over x panels (512-wide). Sweep A: q/k projections
    of group 0 + V projection (all 8 heads, low column half). Sweep B:
    V high half + doubled-angle rotary (R(t)^2 == R(2t)) for group 0.
  - phases 1..3: q/k projections + rotary of group g interleaved with
    attention of head pair g-1. Attention is computed transposed (ST[s,t])
    so exp(ST) feeds the PV matmul directly with v stationary.
  - softmax denominator: DVE accumulates the bf16 exp chunks into an f32r
    panel accumulator; ONE all-ones matmul per (head, panel) does the
    partition reduce + broadcast (vs one matmul per chunk).
  - y stays resident in SBUF (no DRAM spill); the output projection reads
    it directly, interleaved with the last attention pair.
"""

import os
import sys

for _p in ("/opt/trn_rl_repo", "/root/.axon_site/_ro/trn_rl_repo"):
    if os.path.isdir(_p) and _p not in sys.path:
        sys.path.insert(0, _p)

import numpy as np

import concourse.bass as bass
import concourse.mybir as mybir
from concourse import bacc
from concourse.bass import ds
from concourse.tile import TileContext
from concourse.bass_utils import run_bass_kernel_spmd

F32 = mybir.dt.float32
F32R = mybir.dt.float32r
BF16 = mybir.dt.bfloat16
FP16 = mybir.dt.float16

P = 128          # partitions / head dim
T = 2048         # sequence length
E = 2048         # embedding dim
B = 4
HPC = 8          # heads per core
D = 128          # head dim
PAN = 512        # panel width (PSUM bank limit for fp32)
NPAN = T // PAN  # 4
EO = E // P      # 16 contraction chunks for projections
EQ = 4           # eo chunks per input-DMA quarter
NGRP = 4         # head pairs per core
NCH = T // P     # 16 s-chunks (also v t-tiles)
SCALE = 1.0 / float(np.sqrt(D))
NEG = -1.0e9

ADD = mybir.AluOpType.add
MULT = mybir.AluOpType.mult
EXP = mybir.ActivationFunctionType.Exp


def _zip_emit(*lists):
    """Emit thunks from several lists round-robin, proportionally."""
    lists = [list(l) for l in lists if l]
    if not lists:
        return
    total = max(len(l) for l in lists)
    idx = [0.0] * len(lists)
    step = [len(l) / total for l in lists]
    for _ in range(total):
        for li, l in enumerate(lists):
            idx[li] += step[li]
            while idx[li] >= 1.0 and l:
                l.pop(0)()
                idx[li] -= 1.0
    for l in lists:
        for f in l:
            f()


class Ctx:
    pass


def _dma_quarters(nc, dst, src_re, eng=None):
    """Split a [P, EO, W] load into EO/EQ quarter DMAs for early starts."""
    eng = eng if eng is not None else nc.sync
    for qq in range(EO // EQ):
        eng.dma_start(
            dst[:, ds(qq * EQ, EQ), :], src_re[:, ds(qq * EQ, EQ), :]
        )


def build_program():
    nc = bacc.Bacc()
    cx = Ctx()
    cx.nc = nc

    cx.xT = nc.declare_dram_parameter("xT", [E, T], BF16, isOutput=False)
    cx.wqT = nc.declare_dram_parameter("wqT", [E, HPC * D], BF16, isOutput=False)
    cx.wkT = nc.declare_dram_parameter("wkT", [E, HPC * D], BF16, isOutput=False)
    cx.wvT = nc.declare_dram_parameter("wvT", [E, HPC * D], BF16, isOutput=False)
    cx.woT = nc.declare_dram_parameter("woT", [HPC * D, E], BF16, isOutput=False)
    cx.cos2 = nc.declare_dram_parameter("cos2", [P, T], FP16, isOutput=False)
    cx.sin2 = nc.declare_dram_parameter("sin2", [P, T], FP16, isOutput=False)
    cx.mask = nc.declare_dram_parameter("mask", [P, P], BF16, isOutput=False)
    cx.out = nc.declare_dram_parameter("out", [E, T], F32, isOutput=True)

    with TileContext(nc) as tc:
        cx.tc = tc
        with tc.tile_pool(name="const", bufs=1) as cpool:
            om_f = cpool.tile([P, P], F32, tag="om_f")
            nc.vector.memset(om_f, 1.0)
            cx.onesmat = cpool.tile([P, P], FP16, tag="onesmat")
            nc.scalar.copy(cx.onesmat, om_f)
            cx.mk = cpool.tile([P, P], BF16, tag="mk")

            with (
                tc.tile_pool(name="ex", bufs=6) as expool,
                tc.tile_pool(name="acc", bufs=3) as accpool,
                tc.tile_pool(name="dn", bufs=2) as dnpool,
                tc.tile_pool(name="qk", bufs=2) as qkpool,
                tc.tile_pool(name="vp", bufs=1) as vpool,
                tc.tile_pool(name="yp", bufs=1) as ypool,
                tc.tile_pool(name="psS", bufs=3, space="PSUM") as psS,
                tc.tile_pool(name="psY", bufs=2, space="PSUM") as psY,
            ):
                cx.expool, cx.accpool, cx.dnpool = expool, accpool, dnpool
                cx.qkpool = qkpool
                cx.v_sb = vpool.tile([P, NCH, HPC * D], BF16, tag="v")
                cx.y_sb = ypool.tile([P, HPC, T], BF16, tag="y")
                cx.psS, cx.psY = psS, psY
                cx.qkv = {}      # g -> (qT, kT)
                cx._w = {}       # g -> (wq_sb, wk_sb)
                cx._pstate = {}  # g -> {xj: xp tile}

                with (
                    tc.tile_pool(name="tab", bufs=1) as tabpool,
                    tc.tile_pool(name="xp", bufs=2) as xpool,
                    tc.tile_pool(name="wqk", bufs=2) as wqkpool,
                    tc.tile_pool(name="wv", bufs=1) as wvpool,
                    tc.tile_pool(name="rot", bufs=2) as rotpool,
                    tc.tile_pool(name="sw", bufs=4) as swpool,
                    tc.tile_pool(name="psP", bufs=3, space="PSUM") as psP,
                ):
                    cx.xpool, cx.wqkpool, cx.wvpool = xpool, wqkpool, wvpool
                    cx.rotpool, cx.swpool, cx.psP = rotpool, swpool, psP

                    # phase 0 sweep A: q/k of group 0 + v low half
                    for f in _proj_thunks(cx, 0, v_half=0):
                        f()

                    def load_tables():
                        cx.c2 = tabpool.tile([P, T], FP16, tag="c2")
                        nc.gpsimd.dma_start(cx.c2, cx.cos2[:, :])
                        cx.s2 = tabpool.tile([P, T], FP16, tag="s2")
                        nc.gpsimd.dma_start(cx.s2, cx.sin2[:, :])
                        nc.gpsimd.dma_start(cx.mk, cx.mask[:, :])

                    # phase 0 sweep B: v high half + rotary of group 0,
                    # with group 1's weights/panel prefetched behind it.
                    # Tables are emitted after sweep B's wv/x loads so the
                    # gpsimd DMA queue serves the v matmuls first.
                    vs = _vsweep_thunks(cx, v_half=1)
                    vs[0]()
                    vs[1]()
                    load_tables()
                    _zip_emit(
                        vs[2:] + _proj_prefetch(cx, 1),
                        _rot_thunks(cx, 0),
                    )

                    # phases 1..3 merged into ONE proportional zip so the
                    # scheduler always has projection matmuls available to
                    # hide the exp (ACT) latency of attention chunks — the
                    # per-phase version starved at every phase tail.
                    projall = []
                    attnall = []
                    for g in range(1, NGRP):
                        projall += _proj_thunks(cx, g) + _rot_thunks(cx, g)
                        if g + 1 < NGRP:
                            projall += _proj_prefetch(cx, g + 1)
                        attnall += _attn_thunks(cx, g - 1)
                    _zip_emit(projall, attnall)

                with (
                    tc.tile_pool(name="wo", bufs=1) as wopool,
                    tc.tile_pool(name="ob", bufs=3) as opool,
                    tc.tile_pool(name="psO", bufs=3, space="PSUM") as psO,
                ):
                    cx.opool, cx.psO = opool, psO
                    cx.wo_sb = wopool.tile([P, HPC, E], BF16, tag="wo")
                    # quartered, low e-columns first, so the first outproj
                    # e-tiles only wait on the first 1MB
                    for qq in range(4):
                        nc.gpsimd.dma_start(
                            cx.wo_sb[:, :, ds(qq * (E // 4), E // 4)],
                            cx.woT.rearrange("(c p) e -> p c e", p=P)[
                                :, :, ds(qq * (E // 4), E // 4)
                            ],
                        )
                    panels = [_attn_thunks(cx, NGRP - 1, only_jp=jp)
                              for jp in range(NPAN)]
                    oproj = [_outproj_thunks(cx, jp) for jp in range(NPAN)]
                    for f in panels[0]:
                        f()
                    for jp in range(1, NPAN):
                        _zip_emit(panels[jp], oproj[jp - 1])
                    for f in oproj[NPAN - 1]:
                        f()

    nc.finalize()
    return nc


def _load_panel(cx, xj, state):
    def f():
        xp = cx.xpool.tile([P, EO, PAN], BF16, tag="xp")
        _dma_quarters(
            cx.nc, xp,
            cx.xT.rearrange("(eo p) t -> p eo t", p=P)[:, :, ds(xj * PAN, PAN)],
        )
        state[xj] = xp
    return f


def _first_panel_interleaved(cx, g, state):
    """Phase-0 preamble: per-eo DMAs of x panel 0 interleaved with the
    q/k weight chunks so the first matmul chain starts within a few us."""
    nc = cx.nc
    xp = cx.xpool.tile([P, EO, PAN], BF16, tag="xp")
    state[0] = xp
    xsrc = cx.xT.rearrange("(eo p) t -> p eo t", p=P)[:, :, ds(0, PAN)]
    wq_sb = cx.wqkpool.tile([P, EO, 2 * D], BF16, tag="wq")
    wk_sb = cx.wqkpool.tile([P, EO, 2 * D], BF16, tag="wk")
    qsrc = cx.wqT.rearrange("(eo p) d -> p eo d", p=P)
    ksrc = cx.wkT.rearrange("(eo p) d -> p eo d", p=P)
    for eo in range(EO):
        nc.sync.dma_start(xp[:, ds(eo, 1), :], xsrc[:, ds(eo, 1), :])
        nc.gpsimd.dma_start(
            wq_sb[:, ds(eo, 1), :], qsrc[:, ds(eo, 1), ds(g * 2 * D, 2 * D)]
        )
        nc.gpsimd.dma_start(
            wk_sb[:, ds(eo, 1), :], ksrc[:, ds(eo, 1), ds(g * 2 * D, 2 * D)]
        )
    qT = cx.qkpool.tile([P, 2, T], BF16, tag="qT")
    kT = cx.qkpool.tile([P, 2, T], BF16, tag="kT")
    cx.qkv[g] = (qT, kT)
    cx._w[g] = (wq_sb, wk_sb)


def _load_wv_half(cx, half):
    def f():
        wv_sb = cx.wvpool.tile([P, EO, HPC * D // 2], BF16, tag="wv")
        _dma_quarters(
            cx.nc, wv_sb,
            cx.wvT.rearrange("(eo p) d -> p eo d", p=P)[
                :, :, ds(half * HPC * D // 2, HPC * D // 2)
            ],
            eng=cx.nc.gpsimd,
        )
        cx._wv = wv_sb
    return f


def _v_group(cx, state, xj, tt, half):
    """v for all 8 heads, one s-chunk, one 512-column half."""
    def f():
        nc = cx.nc
        xp = state[xj]
        ps = cx.psP.tile([P, PAN], F32, tag="psP")
        for eo in range(EO):
            nc.tensor.matmul(
                ps,
                lhsT=xp[:, eo, ds(tt * P, P)],
                rhs=cx._wv[:, eo, :],
                start=(eo == 0),
                stop=(eo == EO - 1),
            )
        nc.scalar.copy(
            cx.v_sb[:, xj * (PAN // P) + tt, ds(half * PAN, PAN)], ps
        )
    return f


def _proj_prefetch(cx, g):
    """Prefetch thunk for group g's weights + first x panel; emitted during
    the PREVIOUS phase so phase g starts with data in flight (the DMAs wait
    on buffer-slot semaphores, so early emission is always safe)."""
    nc = cx.nc
    state = cx._pstate.setdefault(g, {})

    def f():
        wq_sb = cx.wqkpool.tile([P, EO, 2 * D], BF16, tag="wq")
        _dma_quarters(
            nc, wq_sb,
            cx.wqT.rearrange("(eo p) d -> p eo d", p=P)[:, :, ds(g * 2 * D, 2 * D)],
            eng=nc.gpsimd,
        )
        wk_sb = cx.wqkpool.tile([P, EO, 2 * D], BF16, tag="wk")
        _dma_quarters(
            nc, wk_sb,
            cx.wkT.rearrange("(eo p) d -> p eo d", p=P)[:, :, ds(g * 2 * D, 2 * D)],
            eng=nc.gpsimd,
        )
        qT = cx.qkpool.tile([P, 2, T], BF16, tag="qT")
        kT = cx.qkpool.tile([P, 2, T], BF16, tag="kT")
        cx.qkv[g] = (qT, kT)
        cx._w[g] = (wq_sb, wk_sb)
        _load_panel(cx, 0, state)()

    return [f]


def _proj_thunks(cx, g, v_half=None):
    """Thunks for group g's q/k projections (+ v half during phase 0).

    For g == 0 the weights/panel-0 setup is emitted inline (interleaved
    per-eo DMAs); for g > 0 it happened in _proj_prefetch during the
    previous phase."""
    nc = cx.nc
    thunks = []
    state = cx._pstate.setdefault(g, {})

    if g == 0:
        thunks.append(lambda: _first_panel_interleaved(cx, g, state))
        if v_half is not None:
            thunks.append(_load_wv_half(cx, v_half))

    def qk_group(xj, wi, hl):
        def f():
            xp = state[xj]
            w_sb = cx._w[g][wi]
            dst = cx.qkv[g][wi]
            ps = cx.psP.tile([P, PAN], F32, tag="psP")
            for eo in range(EO):
                nc.tensor.matmul(
                    ps,
                    lhsT=w_sb[:, eo, ds(hl * D, D)],
                    rhs=xp[:, eo, :],
                    start=(eo == 0),
                    stop=(eo == EO - 1),
                )
            nc.vector.tensor_copy(dst[:, hl, ds(xj * PAN, PAN)], ps)
        return f

    for xj in range(NPAN):
        if xj + 1 < NPAN:
            thunks.append(_load_panel(cx, xj + 1, state))
        for wi in range(2):
            for hl in range(2):
                thunks.append(qk_group(xj, wi, hl))
        if v_half is not None:
            for tt in range(PAN // P):
                thunks.append(_v_group(cx, state, xj, tt, v_half))
    return thunks


def _vsweep_thunks(cx, v_half):
    """Second phase-0 sweep: reload x panels, compute the other v half."""
    thunks = []
    state = {}
    thunks.append(_load_wv_half(cx, v_half))
    thunks.append(_load_panel(cx, 0, state))
    for xj in range(NPAN):
        if xj + 1 < NPAN:
            thunks.append(_load_panel(cx, xj + 1, state))
        for tt in range(PAN // P):
            thunks.append(_v_group(cx, state, xj, tt, v_half))
    return thunks


def _rot_thunks(cx, g):
    """Doubled-angle rotary on group g's qT/kT, one 512-panel at a time."""
    nc = cx.nc
    thunks = []

    def rot_panel(src_i, hl, jp):
        def f():
            src = cx.qkv[g][src_i]
            sl = ds(jp * PAN, PAN)
            qsw = cx.swpool.tile([P, PAN], BF16, tag="qsw")
            nc.sync.dma_start(qsw[0:64, :], src[64:128, hl, sl])
            nc.sync.dma_start(qsw[64:128, :], src[0:64, hl, sl])
            tmp = cx.rotpool.tile([P, PAN], FP16, tag="rtmp")
            nc.vector.tensor_tensor(tmp, qsw[:, :], cx.s2[:, sl], op=MULT)
            nc.vector.tensor_tensor(
                src[:, hl, sl], src[:, hl, sl], cx.c2[:, sl], op=MULT
            )
            nc.vector.tensor_tensor(src[:, hl, sl], src[:, hl, sl], tmp, op=ADD)
        return f

    for jp in range(NPAN):
        for src_i in range(2):
            for hl in range(2):
                thunks.append(rot_panel(src_i, hl, jp))
    return thunks


def _attn_thunks(cx, g, only_jp=None):
    """Thunk list for the attention of head pair g (heads 2g, 2g+1)."""
    nc = cx.nc
    thunks = []
    st8 = cx.__dict__.setdefault(f"_attn_state_{g}", {})

    def chunk(hl, jp, i):
        def f():
            qT, kT = cx.qkv[g]
            nch = 4 * jp + 4
            if i == 0:
                ytp = cx.psY.tile([P, PAN], F32, tag="psY")
                acc = cx.accpool.tile([P, PAN], FP16, tag="acc")
                st8[(hl, jp)] = (ytp, acc)
            ytp, acc = st8[(hl, jp)]
            di = i - 4 * jp
            off = P * di if di > 0 else 0
            w = PAN - off
            st = cx.psS.tile([P, PAN], F32, tag="psS")
            stw = st[:, off:PAN]
            nc.tensor.matmul(
                stw,
                lhsT=kT[:, hl, ds(i * P, P)],
                rhs=qT[:, hl, ds(jp * PAN + off, w)],
                start=True,
                stop=True,
            )
            if di >= 0:
                nc.vector.tensor_tensor(
                    st[:, off:off + P], st[:, off:off + P], cx.mk, op=ADD
                )
            ex = cx.expool.tile([P, PAN], BF16, tag="ex")
            exw = ex[:, off:PAN]
            nc.scalar.activation(exw, stw, EXP, scale=SCALE)
            last = i == nch - 1
            nc.tensor.matmul(
                ytp[:, off:PAN],
                lhsT=cx.v_sb[:, i, ds((2 * g + hl) * D, D)],
                rhs=exw,
                start=(i == 0),
                stop=last,
            )
            if i == 0:
                nc.vector.tensor_copy(acc, ex)
            else:
                nc.vector.tensor_tensor(
                    acc[:, off:PAN], acc[:, off:PAN], exw, op=ADD
                )
        return f

    def finalize(hl, jp):
        def f():
            h = 2 * g + hl
            ytp, acc = st8.pop((hl, jp))
            dps = cx.psS.tile([P, PAN], F32, tag="psS")
            nc.tensor.matmul(
                dps, lhsT=cx.onesmat, rhs=acc, start=True, stop=True
            )
            rdb = cx.dnpool.tile([P, PAN], F32, tag="rdb")
            nc.vector.reciprocal_approx_fast(out=rdb, in_=dps)
            nc.vector.tensor_tensor(
                cx.y_sb[:, h, ds(jp * PAN, PAN)], ytp, rdb, op=MULT
            )
        return f

    jps = range(NPAN) if only_jp is None else [only_jp]
    for jp in jps:
        nch = 4 * jp + 4
        for i in range(nch):
            for hl in range(2):
                thunks.append(chunk(hl, jp, i))
        for hl in range(2):
            thunks.append(finalize(hl, jp))
    return thunks


def _outproj_thunks(cx, jp):
    """Output projection for t-panel jp over all 16 e-tiles."""
    nc = cx.nc
    thunks = []

    def etile(et):
        def f():
            ps = cx.psO.tile([P, PAN], F32, tag="psO")
            for dc in range(HPC):
                nc.tensor.matmul(
                    ps,
                    lhsT=cx.wo_sb[:, dc, ds(et * P, P)],
                    rhs=cx.y_sb[:, dc, ds(jp * PAN, PAN)],
                    start=(dc == 0),
                    stop=(dc == HPC - 1),
                )
            ob = cx.opool.tile([P, PAN], F32, tag="ob")
            nc.scalar.copy(ob, ps)
            eng = cx.nc.gpsimd if et % 2 == 0 else cx.nc.scalar
            eng.dma_start(
                cx.out[ds(et * P, P), ds(jp * PAN, PAN)], ob
            )
        return f

    for et in range(2 * HPC):
        thunks.append(etile(et))
    return thunks


def make_tables():
    j = np.arange(0, D, 2, dtype=np.float64) / D
    inv_freq = 1.0 / (10000.0 ** j)
    t = np.arange(T, dtype=np.float64)
    fr = np.outer(t, inv_freq)                            # [T, 64]
    c2 = np.cos(2.0 * fr).T                               # [64, T]
    s2 = np.sin(2.0 * fr).T
    cos2 = np.concatenate([c2, c2], axis=0).astype(np.float16)
    sin2 = np.concatenate([s2, -s2], axis=0).astype(np.float16)
    return cos2, sin2


def make_mask():
    import ml_dtypes
    s = np.arange(P)[:, None]
    c = np.arange(P)[None, :]
    return np.where(s <= c, 0.0, NEG).astype(ml_dtypes.bfloat16)


def make_in_maps(x, wq, wk, wv, wo):
    import ml_dtypes
    bf = ml_dtypes.bfloat16
    cos2, sin2 = make_tables()
    mask = make_mask()
    in_maps = []
    for c in range(8):
        b, hh = c // 2, c % 2
        rows = slice(hh * HPC * D, (hh + 1) * HPC * D)
        in_maps.append({
            "xT": np.ascontiguousarray(x[b].T).astype(bf),
            "wqT": np.ascontiguousarray(wq[rows].T).astype(bf),
            "wkT": np.ascontiguousarray(wk[rows].T).astype(bf),
            "wvT": np.ascontiguousarray(wv[rows].T).astype(bf),
            "woT": np.ascontiguousarray(wo[:, rows].T).astype(bf),
            "cos2": cos2,
            "sin2": sin2,
            "mask": mask,
        })
    return in_maps


_PROGRAM_CACHE = {}


def get_program():
    if "nc" not in _PROGRAM_CACHE:
        _PROGRAM_CACHE["nc"] = build_program()
    return _PROGRAM_CACHE["nc"]


def kernel(x, wq, wk, wv, wo, _results_hook=None):
    x = np.asarray(x, dtype=np.float32)
    wq = np.asarray(wq, dtype=np.float32)
    wk = np.asarray(wk, dtype=np.float32)
    wv = np.asarray(wv, dtype=np.float32)
    wo = np.asarray(wo, dtype=np.float32)

    nc = get_program()
    in_maps = make_in_maps(x, wq, wk, wv, wo)
    res = run_bass_kernel_spmd(nc, in_maps, list(range(8)))
    if _results_hook is not None:
        _results_hook(res)
    outs = [r["out"] for r in res.results]
    full = np.empty((B, T, E), dtype=np.float32)
    for b in range(B):
        full[b] = (outs[2 * b] + outs[2 * b + 1]).T
    return full


# revision 26
# speedup vs baseline: 1.1813x; 1.0056x over previous
"""Trainium2 Bass kernel for causal self-attention with doubled rotary.

Full-input contract: kernel(**inputs) takes the complete tensors
(x [4,2048,2048], wq/wk/wv/wo [2048,2048]) and returns [4,2048,2048] fp32.

Sharding: 8 cores = 4 batch elements x 2 head-halves (8 heads each).
Each core computes a partial output projection (its heads' columns of wo);
the host sums the two partials per batch element.

All matmul operands are bf16 (fp8 exceeds the error budget on every path —
measured 2.8-4.7e-2 vs the 2e-2 gate; all-bf16 lands at ~3.6e-3). bf16
halves DMA bytes vs fp32r and enables FWL weight loads.

Per-core structure (engine streams execute in emission order; independent
work is interleaved at emission time to keep the PE dense):
  - phase 0: two sweeps over x panels (512-wide). Sweep A: q/k projections
    of group 0 + V projection (all 8 heads, low column half). Sweep B:
    V high half + doubled-angle rotary (R(t)^2 == R(2t)) for group 0.
  - phases 1..3: q/k projections + rotary of group g interleaved with
    attention of head pair g-1. Attention is computed transposed (ST[s,t])
    so exp(ST) feeds the PV matmul directly with v stationary.
  - softmax denominator: DVE accumulates the bf16 exp chunks into an f32r
    panel accumulator; ONE all-ones matmul per (head, panel) does the
    partition reduce + broadcast (vs one matmul per chunk).
  - y stays resident in SBUF (no DRAM spill); the output projection reads
    it directly, interleaved with the last attention pair.
"""

import os
import sys

for _p in ("/opt/trn_rl_repo", "/root/.axon_site/_ro/trn_rl_repo"):
    if os.path.isdir(_p) and _p not in sys.path:
        sys.path.insert(0, _p)

import numpy as np

import concourse.bass as bass
import concourse.mybir as mybir
from concourse import bacc
from concourse.bass import ds
from concourse.tile import TileContext
from concourse.bass_utils import run_bass_kernel_spmd

F32 = mybir.dt.float32
F32R = mybir.dt.float32r
BF16 = mybir.dt.bfloat16
FP16 = mybir.dt.float16

P = 128          # partitions / head dim
T = 2048         # sequence length
E = 2048         # embedding dim
B = 4
HPC = 8          # heads per core
D = 128          # head dim
PAN = 512        # panel width (PSUM bank limit for fp32)
NPAN = T // PAN  # 4
EO = E // P      # 16 contraction chunks for projections
EQ = 4           # eo chunks per input-DMA quarter
NGRP = 4         # head pairs per core
NCH = T // P     # 16 s-chunks (also v t-tiles)
SCALE = 1.0 / float(np.sqrt(D))
NEG = -1.0e9

ADD = mybir.AluOpType.add
MULT = mybir.AluOpType.mult
EXP = mybir.ActivationFunctionType.Exp


def _zip_emit(*lists):
    """Emit thunks from several lists round-robin, proportionally."""
    lists = [list(l) for l in lists if l]
    if not lists:
        return
    total = max(len(l) for l in lists)
    idx = [0.0] * len(lists)
    step = [len(l) / total for l in lists]
    for _ in range(total):
        for li, l in enumerate(lists):
            idx[li] += step[li]
            while idx[li] >= 1.0 and l:
                l.pop(0)()
                idx[li] -= 1.0
    for l in lists:
        for f in l:
            f()


class Ctx:
    pass


def _dma_quarters(nc, dst, src_re, eng=None):
    """Split a [P, EO, W] load into EO/EQ quarter DMAs for early starts."""
    eng = eng if eng is not None else nc.sync
    for qq in range(EO // EQ):
        eng.dma_start(
            dst[:, ds(qq * EQ, EQ), :], src_re[:, ds(qq * EQ, EQ), :]
        )


def build_program():
    nc = bacc.Bacc()
    cx = Ctx()
    cx.nc = nc

    cx.xT = nc.declare_dram_parameter("xT", [E, T], BF16, isOutput=False)
    cx.wqT = nc.declare_dram_parameter("wqT", [E, HPC * D], BF16, isOutput=False)
    cx.wkT = nc.declare_dram_parameter("wkT", [E, HPC * D], BF16, isOutput=False)
    cx.wvT = nc.declare_dram_parameter("wvT", [E, HPC * D], BF16, isOutput=False)
    cx.woT = nc.declare_dram_parameter("woT", [HPC * D, E], BF16, isOutput=False)
    cx.cos2 = nc.declare_dram_parameter("cos2", [P, T], FP16, isOutput=False)
    cx.sin2 = nc.declare_dram_parameter("sin2", [P, T], FP16, isOutput=False)
    cx.mask = nc.declare_dram_parameter("mask", [P, P], BF16, isOutput=False)
    cx.out = nc.declare_dram_parameter("out", [E, T], F32, isOutput=True)

    with TileContext(nc) as tc:
        cx.tc = tc
        with tc.tile_pool(name="const", bufs=1) as cpool:
            om_f = cpool.tile([P, P], F32, tag="om_f")
            nc.vector.memset(om_f, 1.0)
            cx.onesmat = cpool.tile([P, P], FP16, tag="onesmat")
            nc.scalar.copy(cx.onesmat, om_f)
            cx.mk = cpool.tile([P, P], BF16, tag="mk")

            with (
                tc.tile_pool(name="ex", bufs=6) as expool,
                tc.tile_pool(name="acc", bufs=3) as accpool,
                tc.tile_pool(name="dn", bufs=2) as dnpool,
                tc.tile_pool(name="qk", bufs=2) as qkpool,
                tc.tile_pool(name="vp", bufs=1) as vpool,
                tc.tile_pool(name="yp", bufs=1) as ypool,
                tc.tile_pool(name="psS", bufs=3, space="PSUM") as psS,
                tc.tile_pool(name="psY", bufs=2, space="PSUM") as psY,
            ):
                cx.expool, cx.accpool, cx.dnpool = expool, accpool, dnpool
                cx.qkpool = qkpool
                cx.v_sb = vpool.tile([P, NCH, HPC * D], BF16, tag="v")
                cx.y_sb = ypool.tile([P, HPC, T], BF16, tag="y")
                cx.psS, cx.psY = psS, psY
                cx.qkv = {}      # g -> (qT, kT)
                cx._w = {}       # g -> (wq_sb, wk_sb)
                cx._pstate = {}  # g -> {xj: xp tile}

                with (
                    tc.tile_pool(name="tab", bufs=1) as tabpool,
                    tc.tile_pool(name="xp", bufs=2) as xpool,
                    tc.tile_pool(name="wqk", bufs=2) as wqkpool,
                    tc.tile_pool(name="wv", bufs=1) as wvpool,
                    tc.tile_pool(name="rot", bufs=2) as rotpool,
                    tc.tile_pool(name="sw", bufs=4) as swpool,
                    tc.tile_pool(name="psP", bufs=3, space="PSUM") as psP,
                ):
                    cx.xpool, cx.wqkpool, cx.wvpool = xpool, wqkpool, wvpool
                    cx.rotpool, cx.swpool, cx.psP = rotpool, swpool, psP

                    # phase 0 sweep A: q/k of group 0 + v low half
                    for f in _proj_thunks(cx, 0, v_half=0):
                        f()

                    def load_tables():
                        cx.c2 = tabpool.tile([P, T], FP16, tag="c2")
                        nc.gpsimd.dma_start(cx.c2, cx.cos2[:, :])
                        cx.s2 = tabpool.tile([P, T], FP16, tag="s2")
                        nc.gpsimd.dma_start(cx.s2, cx.sin2[:, :])
                        nc.gpsimd.dma_start(cx.mk, cx.mask[:, :])

                    # phase 0 sweep B: v high half + rotary of group 0,
                    # with group 1's weights/panel prefetched behind it.
                    # Tables are emitted after sweep B's wv/x loads so the
                    # gpsimd DMA queue serves the v matmuls first.
                    vs = _vsweep_thunks(cx, v_half=1)
                    vs[0]()
                    vs[1]()
                    load_tables()
                    _zip_emit(
                        vs[2:] + _proj_prefetch(cx, 1),
                        _rot_thunks(cx, 0),
                    )

                    # phases 1..3 merged into ONE proportional zip so the
                    # scheduler always has projection matmuls available to
                    # hide the exp (ACT) latency of attention chunks — the
                    # per-phase version starved at every phase tail.
                    projall = []
                    attnall = []
                    for g in range(1, NGRP):
                        projall += _proj_thunks(cx, g) + _rot_thunks(cx, g)
                        if g + 1 < NGRP:
                            projall += _proj_prefetch(cx, g + 1)
                        attnall += _attn_thunks(cx, g - 1)
                    _zip_emit(projall, attnall)

                with (
                    tc.tile_pool(name="wo", bufs=1) as wopool,
                    tc.tile_pool(name="ob", bufs=3) as opool,
                    tc.tile_pool(name="psO", bufs=3, space="PSUM") as psO,
                ):
                    cx.opool, cx.psO = opool, psO
                    cx.wo_sb = wopool.tile([P, HPC, E], BF16, tag="wo")
                    # quartered, low e-columns first, so the first outproj
                    # e-tiles only wait on the first 1MB
                    for qq in range(4):
                        nc.gpsimd.dma_start(
                            cx.wo_sb[:, :, ds(qq * (E // 4), E // 4)],
                            cx.woT.rearrange("(c p) e -> p c e", p=P)[
                                :, :, ds(qq * (E // 4), E // 4)
                            ],
                        )
                    panels = [_attn_thunks(cx, NGRP - 1, only_jp=jp)
                              for jp in range(NPAN)]
                    oproj = [_outproj_thunks(cx, jp) for jp in range(NPAN)]
                    for f in panels[0]:
                        f()
                    for jp in range(1, NPAN):
                        for f in panels[jp][:6]:
                            f()
                        _zip_emit(panels[jp][6:], oproj[jp - 1])
                    for f in oproj[NPAN - 1]:
                        f()

    nc.finalize()
    return nc


def _load_panel(cx, xj, state):
    def f():
        xp = cx.xpool.tile([P, EO, PAN], BF16, tag="xp")
        _dma_quarters(
            cx.nc, xp,
            cx.xT.rearrange("(eo p) t -> p eo t", p=P)[:, :, ds(xj * PAN, PAN)],
        )
        state[xj] = xp
    return f


def _first_panel_interleaved(cx, g, state):
    """Phase-0 preamble: per-eo DMAs of x panel 0 interleaved with the
    q/k weight chunks so the first matmul chain starts within a few us."""
    nc = cx.nc
    xp = cx.xpool.tile([P, EO, PAN], BF16, tag="xp")
    state[0] = xp
    xsrc = cx.xT.rearrange("(eo p) t -> p eo t", p=P)[:, :, ds(0, PAN)]
    wq_sb = cx.wqkpool.tile([P, EO, 2 * D], BF16, tag="wq")
    wk_sb = cx.wqkpool.tile([P, EO, 2 * D], BF16, tag="wk")
    qsrc = cx.wqT.rearrange("(eo p) d -> p eo d", p=P)
    ksrc = cx.wkT.rearrange("(eo p) d -> p eo d", p=P)
    for eo in range(EO):
        nc.sync.dma_start(xp[:, ds(eo, 1), :], xsrc[:, ds(eo, 1), :])
        nc.gpsimd.dma_start(
            wq_sb[:, ds(eo, 1), :], qsrc[:, ds(eo, 1), ds(g * 2 * D, 2 * D)]
        )
        nc.gpsimd.dma_start(
            wk_sb[:, ds(eo, 1), :], ksrc[:, ds(eo, 1), ds(g * 2 * D, 2 * D)]
        )
    qT = cx.qkpool.tile([P, 2, T], BF16, tag="qT")
    kT = cx.qkpool.tile([P, 2, T], BF16, tag="kT")
    cx.qkv[g] = (qT, kT)
    cx._w[g] = (wq_sb, wk_sb)


def _load_wv_half(cx, half):
    def f():
        wv_sb = cx.wvpool.tile([P, EO, HPC * D // 2], BF16, tag="wv")
        _dma_quarters(
            cx.nc, wv_sb,
            cx.wvT.rearrange("(eo p) d -> p eo d", p=P)[
                :, :, ds(half * HPC * D // 2, HPC * D // 2)
            ],
            eng=cx.nc.gpsimd,
        )
        cx._wv = wv_sb
    return f


def _v_group(cx, state, xj, tt, half):
    """v for all 8 heads, one s-chunk, one 512-column half."""
    def f():
        nc = cx.nc
        xp = state[xj]
        ps = cx.psP.tile([P, PAN], F32, tag="psP")
        for eo in range(EO):
            nc.tensor.matmul(
                ps,
                lhsT=xp[:, eo, ds(tt * P, P)],
                rhs=cx._wv[:, eo, :],
                start=(eo == 0),
                stop=(eo == EO - 1),
            )
        nc.scalar.copy(
            cx.v_sb[:, xj * (PAN // P) + tt, ds(half * PAN, PAN)], ps
        )
    return f


def _proj_prefetch(cx, g):
    """Prefetch thunk for group g's weights + first x panel; emitted during
    the PREVIOUS phase so phase g starts with data in flight (the DMAs wait
    on buffer-slot semaphores, so early emission is always safe)."""
    nc = cx.nc
    state = cx._pstate.setdefault(g, {})

    def f():
        wq_sb = cx.wqkpool.tile([P, EO, 2 * D], BF16, tag="wq")
        _dma_quarters(
            nc, wq_sb,
            cx.wqT.rearrange("(eo p) d -> p eo d", p=P)[:, :, ds(g * 2 * D, 2 * D)],
            eng=nc.gpsimd,
        )
        wk_sb = cx.wqkpool.tile([P, EO, 2 * D], BF16, tag="wk")
        _dma_quarters(
            nc, wk_sb,
            cx.wkT.rearrange("(eo p) d -> p eo d", p=P)[:, :, ds(g * 2 * D, 2 * D)],
            eng=nc.gpsimd,
        )
        qT = cx.qkpool.tile([P, 2, T], BF16, tag="qT")
        kT = cx.qkpool.tile([P, 2, T], BF16, tag="kT")
        cx.qkv[g] = (qT, kT)
        cx._w[g] = (wq_sb, wk_sb)
        _load_panel(cx, 0, state)()

    return [f]


def _proj_thunks(cx, g, v_half=None):
    """Thunks for group g's q/k projections (+ v half during phase 0).

    For g == 0 the weights/panel-0 setup is emitted inline (interleaved
    per-eo DMAs); for g > 0 it happened in _proj_prefetch during the
    previous phase."""
    nc = cx.nc
    thunks = []
    state = cx._pstate.setdefault(g, {})

    if g == 0:
        thunks.append(lambda: _first_panel_interleaved(cx, g, state))
        if v_half is not None:
            thunks.append(_load_wv_half(cx, v_half))

    def qk_group(xj, wi, hl):
        def f():
            xp = state[xj]
            w_sb = cx._w[g][wi]
            dst = cx.qkv[g][wi]
            ps = cx.psP.tile([P, PAN], F32, tag="psP")
            for eo in range(EO):
                nc.tensor.matmul(
                    ps,
                    lhsT=w_sb[:, eo, ds(hl * D, D)],
                    rhs=xp[:, eo, :],
                    start=(eo == 0),
                    stop=(eo == EO - 1),
                )
            nc.vector.tensor_copy(dst[:, hl, ds(xj * PAN, PAN)], ps)
        return f

    for xj in range(NPAN):
        if xj + 1 < NPAN:
            thunks.append(_load_panel(cx, xj + 1, state))
        for wi in range(2):
            for hl in range(2):
                thunks.append(qk_group(xj, wi, hl))
        if v_half is not None:
            for tt in range(PAN // P):
                thunks.append(_v_group(cx, state, xj, tt, v_half))
    return thunks


def _vsweep_thunks(cx, v_half):
    """Second phase-0 sweep: reload x panels, compute the other v half."""
    thunks = []
    state = {}
    thunks.append(_load_wv_half(cx, v_half))
    thunks.append(_load_panel(cx, 0, state))
    for xj in range(NPAN):
        if xj + 1 < NPAN:
            thunks.append(_load_panel(cx, xj + 1, state))
        for tt in range(PAN // P):
            thunks.append(_v_group(cx, state, xj, tt, v_half))
    return thunks


def _rot_thunks(cx, g):
    """Doubled-angle rotary on group g's qT/kT, one 512-panel at a time."""
    nc = cx.nc
    thunks = []

    def rot_panel(src_i, hl, jp):
        def f():
            src = cx.qkv[g][src_i]
            sl = ds(jp * PAN, PAN)
            qsw = cx.swpool.tile([P, PAN], BF16, tag="qsw")
            nc.sync.dma_start(qsw[0:64, :], src[64:128, hl, sl])
            nc.sync.dma_start(qsw[64:128, :], src[0:64, hl, sl])
            tmp = cx.rotpool.tile([P, PAN], FP16, tag="rtmp")
            nc.vector.tensor_tensor(tmp, qsw[:, :], cx.s2[:, sl], op=MULT)
            nc.vector.tensor_tensor(
                src[:, hl, sl], src[:, hl, sl], cx.c2[:, sl], op=MULT
            )
            nc.vector.tensor_tensor(src[:, hl, sl], src[:, hl, sl], tmp, op=ADD)
        return f

    for jp in range(NPAN):
        for src_i in range(2):
            for hl in range(2):
                thunks.append(rot_panel(src_i, hl, jp))
    return thunks


def _attn_thunks(cx, g, only_jp=None):
    """Thunk list for the attention of head pair g (heads 2g, 2g+1)."""
    nc = cx.nc
    thunks = []
    st8 = cx.__dict__.setdefault(f"_attn_state_{g}", {})

    def chunk(hl, jp, i):
        def f():
            qT, kT = cx.qkv[g]
            nch = 4 * jp + 4
            if i == 0:
                ytp = cx.psY.tile([P, PAN], F32, tag="psY")
                acc = cx.accpool.tile([P, PAN], FP16, tag="acc")
                st8[(hl, jp)] = (ytp, acc)
            ytp, acc = st8[(hl, jp)]
            di = i - 4 * jp
            off = P * di if di > 0 else 0
            w = PAN - off
            st = cx.psS.tile([P, PAN], F32, tag="psS")
            stw = st[:, off:PAN]
            nc.tensor.matmul(
                stw,
                lhsT=kT[:, hl, ds(i * P, P)],
                rhs=qT[:, hl, ds(jp * PAN + off, w)],
                start=True,
                stop=True,
            )
            if di >= 0:
                nc.vector.tensor_tensor(
                    st[:, off:off + P], st[:, off:off + P], cx.mk, op=ADD
                )
            ex = cx.expool.tile([P, PAN], BF16, tag="ex")
            exw = ex[:, off:PAN]
            nc.scalar.activation(exw, stw, EXP, scale=SCALE)
            last = i == nch - 1
            nc.tensor.matmul(
                ytp[:, off:PAN],
                lhsT=cx.v_sb[:, i, ds((2 * g + hl) * D, D)],
                rhs=exw,
                start=(i == 0),
                stop=last,
            )
            if i == 0:
                nc.vector.tensor_copy(acc, ex)
            else:
                nc.vector.tensor_tensor(
                    acc[:, off:PAN], acc[:, off:PAN], exw, op=ADD
                )
        return f

    def finalize(hl, jp):
        def f():
            h = 2 * g + hl
            ytp, acc = st8.pop((hl, jp))
            dps = cx.psS.tile([P, PAN], F32, tag="psS")
            nc.tensor.matmul(
                dps, lhsT=cx.onesmat, rhs=acc, start=True, stop=True
            )
            rdb = cx.dnpool.tile([P, PAN], F32, tag="rdb")
            nc.vector.reciprocal_approx_fast(out=rdb, in_=dps)
            nc.vector.tensor_tensor(
                cx.y_sb[:, h, ds(jp * PAN, PAN)], ytp, rdb, op=MULT
            )
        return f

    jps = range(NPAN) if only_jp is None else [only_jp]
    for jp in jps:
        nch = 4 * jp + 4
        for i in range(nch):
            for hl in range(2):
                thunks.append(chunk(hl, jp, i))
        for hl in range(2):
            thunks.append(finalize(hl, jp))
    return thunks


def _outproj_thunks(cx, jp):
    """Output projection for t-panel jp over all 16 e-tiles."""
    nc = cx.nc
    thunks = []

    def etile(et):
        def f():
            ps = cx.psO.tile([P, PAN], F32, tag="psO")
            for dc in range(HPC):
                nc.tensor.matmul(
                    ps,
                    lhsT=cx.wo_sb[:, dc, ds(et * P, P)],
                    rhs=cx.y_sb[:, dc, ds(jp * PAN, PAN)],
                    start=(dc == 0),
                    stop=(dc == HPC - 1),
                )
            ob = cx.opool.tile([P, PAN], F32, tag="ob")
            nc.scalar.copy(ob, ps)
            eng = cx.nc.gpsimd if et % 2 == 0 else cx.nc.scalar
            eng.dma_start(
                cx.out[ds(et * P, P), ds(jp * PAN, PAN)], ob
            )
        return f

    for et in range(2 * HPC):
        thunks.append(etile(et))
    return thunks


def make_tables():
    j = np.arange(0, D, 2, dtype=np.float64) / D
    inv_freq = 1.0 / (10000.0 ** j)
    t = np.arange(T, dtype=np.float64)
    fr = np.outer(t, inv_freq)                            # [T, 64]
    c2 = np.cos(2.0 * fr).T                               # [64, T]
    s2 = np.sin(2.0 * fr).T
    cos2 = np.concatenate([c2, c2], axis=0).astype(np.float16)
    sin2 = np.concatenate([s2, -s2], axis=0).astype(np.float16)
    return cos2, sin2


def make_mask():
    import ml_dtypes
    s = np.arange(P)[:, None]
    c = np.arange(P)[None, :]
    return np.where(s <= c, 0.0, NEG).astype(ml_dtypes.bfloat16)


def make_in_maps(x, wq, wk, wv, wo):
    import ml_dtypes
    bf = ml_dtypes.bfloat16
    cos2, sin2 = make_tables()
    mask = make_mask()
    in_maps = []
    for c in range(8):
        b, hh = c // 2, c % 2
        rows = slice(hh * HPC * D, (hh + 1) * HPC * D)
        in_maps.append({
            "xT": np.ascontiguousarray(x[b].T).astype(bf),
            "wqT": np.ascontiguousarray(wq[rows].T).astype(bf),
            "wkT": np.ascontiguousarray(wk[rows].T).astype(bf),
            "wvT": np.ascontiguousarray(wv[rows].T).astype(bf),
            "woT": np.ascontiguousarray(wo[:, rows].T).astype(bf),
            "cos2": cos2,
            "sin2": sin2,
            "mask": mask,
        })
    return in_maps


_PROGRAM_CACHE = {}


def get_program():
    if "nc" not in _PROGRAM_CACHE:
        _PROGRAM_CACHE["nc"] = build_program()
    return _PROGRAM_CACHE["nc"]


def kernel(x, wq, wk, wv, wo, _results_hook=None):
    x = np.asarray(x, dtype=np.float32)
    wq = np.asarray(wq, dtype=np.float32)
    wk = np.asarray(wk, dtype=np.float32)
    wv = np.asarray(wv, dtype=np.float32)
    wo = np.asarray(wo, dtype=np.float32)

    nc = get_program()
    in_maps = make_in_maps(x, wq, wk, wv, wo)
    res = run_bass_kernel_spmd(nc, in_maps, list(range(8)))
    if _results_hook is not None:
        _results_hook(res)
    outs = [r["out"] for r in res.results]
    full = np.empty((B, T, E), dtype=np.float32)
    for b in range(B):
        full[b] = (outs[2 * b] + outs[2 * b + 1]).T
    return full
